# revision 33
# baseline (speedup 1.0000x reference)
"""2-layer GCN (PyG GCNConv semantics) as a Bass/Tile kernel for TRN2. v2.

Math (per GCNConv layer, self-loops added, deg from dst in-degree + 1):
  out[d] = b + sum_{e: dst[e]=d} w[e] * t[src[e]]      with w[e] = rsqrt(deg[src]*deg[dst])
  where t = x        (layer 1: aggregate first, then @W1 — W commutes with aggregation)
        t = y1 @ W2  (layer 2: transform first)

Key performance structure (vs v1 baseline):
  - dma_gather descriptor generation runs on a Q7 core PAIR selected by
    queue_num; round-robin over 4 SWDGE queues gives ~4x parallel desc-gen.
  - all gathers / scatter matmuls in bf16 (x pre-cast to bf16 on host).
  - edge padding uses NEGATIVE indices at the bucket tail: the gather ucode
    trims trailing negatives, skipping their descriptors (incl. per-core
    load imbalance under the shared max-sized NEFF).
  - the t exchange is split into 4 piece-wise AllGathers so most of the
    exchange overlaps layer-1 compute; layer-2 edges are bucketed by
    source piece (int16-indexable piece tables).
"""

import math
import sys

import numpy as np

sys.path.insert(0, "/opt/trn_rl_repo")

import concourse.bass as bass
import concourse.bacc as bacc
import concourse.mybir as mybir
import concourse.tile as tile
from concourse.masks import make_identity

F32 = mybir.dt.float32
BF16 = mybir.dt.bfloat16
I16 = mybir.dt.int16
I32 = mybir.dt.int32

P = 128
QS = 32768  # int16-indexable rows per gather table slice (layer 1)
NEG_SLOPE = 0.01
DW = 256  # dst columns per super (TPS=2 tiles)
TPS = 2
MSG_BUFS = 18
PAD_VALID = True  # True: pad with idx 0 (all lanes gathered; sim-friendly)

F_IN, H1, H2, N_CLS = 128, 180, 120, 16


class Meta:
    pass


def _bucketize(src, dst, n_cores, chunk, NS, q_of, nq, idx_of):
    """Bucket edges by (core, super, q); returns per-core padded arrays + meta.

    idx_of: int16 gather index within table q for each edge.
    """
    P_ = P
    core_of = dst // chunk
    sup_of = (dst % chunk) // DW

    counts = np.zeros((n_cores, NS, nq), dtype=np.int64)
    np.add.at(counts, (core_of, sup_of, q_of), 1)
    mx = counts.max(axis=0)  # [NS, nq]
    slots_sq = ((mx + P_ - 1) // P_).astype(np.int64)

    off = 0
    sp_meta = []
    for sp in range(NS):
        groups = []
        for q in range(nq):
            s = int(slots_sq[sp, q])
            if s == 0:
                continue
            groups.append((q, off, s))
            off += s
        sp_meta.append(groups)
    total_slots = off

    order = np.lexsort((src, q_of, sup_of, core_of))
    d_s = dst[order]
    i_s = idx_of[order]
    keys = ((core_of * NS + sup_of) * nq + q_of)[order]
    bstart = np.searchsorted(keys, np.arange(n_cores * NS * nq), side="left")
    bend = np.searchsorted(keys, np.arange(n_cores * NS * nq), side="right")

    n_buckets = sum(len(g) for g in sp_meta)
    per_core = []
    for k in range(n_cores):
        gflat = np.full(total_slots * P_, 0 if PAD_VALID else -1, dtype=np.int16)
        dflat = np.full(total_slots * P_, 300.0, dtype=np.float32)
        cnts = np.zeros(n_buckets, dtype=np.int32)
        bi = 0
        for sp in range(NS):
            for (q, g0, s) in sp_meta[sp]:
                b = (k * NS + sp) * nq + q
                i0, i1 = bstart[b], bend[b]
                n = i1 - i0
                pos = g0 * P_
                if n == 0:
                    # keep >= 1 valid index per call (gathers row 0; its
                    # dstloc stays at the 999 sentinel so S zeroes it)
                    gflat[pos] = 0
                    n = 1
                else:
                    gflat[pos : pos + n] = i_s[i0:i1]
                    dflat[pos : pos + n] = (d_s[i0:i1] % chunk - sp * DW).astype(
                        np.float32
                    )
                cnts[bi] = s * P_ if PAD_VALID else n
                bi += 1
        import ml_dtypes
        gidx = np.tile(gflat.reshape(-1, 16).T, (8, 1))  # [128, X]
        dstloc = dflat.reshape(-1, P_).T.astype(ml_dtypes.bfloat16)
        per_core.append(dict(gidx=gidx, dstloc=dstloc, cnts=cnts))
    maxnsl = int(slots_sq.max()) if total_slots else 1
    return sp_meta, total_slots, maxnsl, n_buckets, per_core


def prep(edge_index, n_nodes, n_cores):
    src = np.asarray(edge_index[0], dtype=np.int64)
    dst = np.asarray(edge_index[1], dtype=np.int64)
    deg = np.bincount(dst, minlength=n_nodes) + 1
    dinv = (1.0 / np.sqrt(deg.astype(np.float64))).astype(np.float32)

    assert n_nodes % n_cores == 0
    chunk = n_nodes // n_cores
    NT = math.ceil(chunk / P)
    NS = math.ceil(NT / TPS)

    # ---- layer-1 buckets: q = src quarter of the x table
    NQ1 = math.ceil(n_nodes / QS)
    q1 = src // QS
    idx1 = (src - q1 * QS).astype(np.int16)
    sp1, slots1, maxnsl1, nb1, pc1 = _bucketize(
        src, dst, n_cores, chunk, NS, q1, NQ1, idx1
    )

    # ---- layer-2 buckets: q = source piece (tile-aligned ranges of each chunk)
    npc = 4 if NT >= 4 else 1
    base = NT // npc
    piece_tiles = [base + (1 if i < NT % npc else 0) for i in range(npc)]
    assert sum(piece_tiles) == NT
    pstart = [0]
    for t in piece_tiles[:-1]:
        pstart.append(pstart[-1] + t * P)
    prow = []
    for i, t in enumerate(piece_tiles):
        hi = min(chunk, pstart[i] + t * P)
        prow.append(hi - pstart[i])
    NQ2 = len(piece_tiles)
    s_core = src // chunk
    s_loc = src % chunk
    q2 = np.searchsorted(np.array(pstart[1:]), s_loc, side="right")
    idx2 = (
        s_core * np.array(prow)[q2] + (s_loc - np.array(pstart)[q2])
    ).astype(np.int16)
    sp2, slots2, maxnsl2, nb2, pc2 = _bucketize(
        src, dst, n_cores, chunk, NS, q2, NQ2, idx2
    )

    per_core = []
    for k in range(n_cores):
        dv = np.ones(NT * P, dtype=np.float32)
        dv[:chunk] = dinv[k * chunk : (k + 1) * chunk]
        dinvt = dv.reshape(NT, P).T.copy()  # [128, NT] per-node 1/sqrt(deg)
        import ml_dtypes
        dinvrep = np.broadcast_to(
            dv.reshape(1, NT * P), (P, NT * P)
        ).astype(ml_dtypes.bfloat16)  # [128, NT*128] replicated rows
        sq = np.ones(NT * P, dtype=np.float64)
        sq[:chunk] = np.sqrt(deg[k * chunk : (k + 1) * chunk])
        sqdrow = sq.reshape(1, NT * P).astype(ml_dtypes.bfloat16)
        per_core.append(
            dict(
                gidx1=pc1[k]["gidx"], dstloc1=pc1[k]["dstloc"],
                gidx2=pc2[k]["gidx"], dstloc2=pc2[k]["dstloc"],
                gcnt1=pc1[k]["cnts"].reshape(1, -1),
                gcnt2=pc2[k]["cnts"].reshape(1, -1),
                dinvt=dinvt, dinvrep=dinvrep, sqdrow=sqdrow,
            )
        )
    m_dinv = dinv

    m = Meta()
    m.n_nodes = n_nodes
    m.n_cores = n_cores
    m.chunk = chunk
    m.NT = NT
    m.NS = NS
    m.sp1, m.slots1, m.maxnsl1, m.nb1 = sp1, slots1, maxnsl1, nb1
    m.sp2, m.slots2, m.maxnsl2, m.nb2 = sp2, slots2, maxnsl2, nb2
    m.NQ1, m.NQ2 = NQ1, NQ2
    m.qbounds1 = [(q * QS, min(n_nodes, (q + 1) * QS)) for q in range(NQ1)]
    m.piece_tiles = piece_tiles
    m.pstart = pstart
    m.prow = prow
    m.dinv = m_dinv
    return m, per_core


def build(m: Meta):
    nc = bacc.Bacc(
        trn_type="TRN2",
        num_devices=m.n_cores,
        target_bir_lowering=False,
        num_swdge_queues=4,
    )
    chunk, NT, NS = m.chunk, m.NT, m.NS

    xbf_d = nc.dram_tensor("xbf", [m.n_nodes, P], BF16, kind="ExternalInput")
    xownb_d = nc.dram_tensor("xownb", [chunk, P], BF16, kind="ExternalInput")
    w1_d = nc.dram_tensor("W1", [F_IN, H1], F32, kind="ExternalInput")
    b1_d = nc.dram_tensor("b1", [H1, 1], F32, kind="ExternalInput")
    b1row_d = nc.dram_tensor("b1row", [1, H1], BF16, kind="ExternalInput")
    b2row_d = nc.dram_tensor("b2row", [1, H2], BF16, kind="ExternalInput")
    w2_d = nc.dram_tensor("W2", [H1, H2], F32, kind="ExternalInput")
    b2_d = nc.dram_tensor("b2", [H2, 1], F32, kind="ExternalInput")
    wl_d = nc.dram_tensor("Wl", [H2, N_CLS], F32, kind="ExternalInput")
    bl_d = nc.dram_tensor("bl", [1, N_CLS], F32, kind="ExternalInput")
    gidx1_d = nc.dram_tensor("gidx1", [P, m.slots1 * 8], I16, kind="ExternalInput")
    dstloc1_d = nc.dram_tensor("dstloc1", [P, m.slots1], BF16, kind="ExternalInput")
    gidx2_d = nc.dram_tensor("gidx2", [P, m.slots2 * 8], I16, kind="ExternalInput")
    dstloc2_d = nc.dram_tensor("dstloc2", [P, m.slots2], BF16, kind="ExternalInput")
    gcnt1_d = nc.dram_tensor("gcnt1", [1, m.nb1], I32, kind="ExternalInput")
    gcnt2_d = nc.dram_tensor("gcnt2", [1, m.nb2], I32, kind="ExternalInput")
    dinvt_d = nc.dram_tensor("dinvt", [P, NT], F32, kind="ExternalInput")
    dinvrep_d = nc.dram_tensor("dinvrep", [P, NT * P], BF16, kind="ExternalInput")
    sqdrow_d = nc.dram_tensor("sqdrow", [1, NT * P], BF16, kind="ExternalInput")
    out_d = nc.dram_tensor("out", [chunk, N_CLS], F32, kind="ExternalOutput")

    # piece-wise t exchange tensors
    tchunk_p = [
        nc.dram_tensor(f"tchunk{p}", [m.prow[p], P], BF16, kind="Internal")
        for p in range(m.NQ2)
    ]
    tfull_p = [
        nc.dram_tensor(
            f"tfull{p}", [m.n_cores * m.prow[p], P], BF16, kind="Internal",
            addr_space="Shared",
        )
        for p in range(m.NQ2)
    ]

    # tile -> (piece, local row start)
    tile_piece = []
    for t in range(NT):
        acc_t = 0
        for p, pt in enumerate(m.piece_tiles):
            if t < acc_t + pt:
                tile_piece.append((p, (t - acc_t) * P))
                break
            acc_t += pt

    from contextlib import ExitStack

    qctr = [0]

    def next_q():
        q = qctr[0] % 4
        qctr[0] += 1
        return q

    with tile.TileContext(nc) as tc, ExitStack() as ctx:
        cpool = ctx.enter_context(tc.tile_pool(name="consts", bufs=1))
        mpool = ctx.enter_context(tc.tile_pool(name="msg", bufs=MSG_BUFS))
        spool = ctx.enter_context(tc.tile_pool(name="onehot", bufs=18))
        wkpool = ctx.enter_context(tc.tile_pool(name="work", bufs=3))
        scat_pp = ctx.enter_context(tc.tile_pool(name="scat", bufs=2, space="PSUM"))
        y1_pp = ctx.enter_context(tc.tile_pool(name="y1ps", bufs=1, space="PSUM"))
        t_pp = ctx.enter_context(tc.tile_pool(name="tps", bufs=2, space="PSUM"))
        log_pp = ctx.enter_context(tc.tile_pool(name="logps", bufs=2, space="PSUM"))

        # ---- constants / resident tiles
        w1b_s = cpool.tile([F_IN, H1], BF16)
        w2ab_s = cpool.tile([P, H2], BF16)
        w2bb_s = cpool.tile([H1 - P, H2], BF16)
        wlb_s = cpool.tile([H2, N_CLS], BF16)
        bl_s = cpool.tile([1, N_CLS], F32)
        b1a_s = cpool.tile([P, 1], F32)
        b1b_s = cpool.tile([H1 - P, 1], F32)
        b2_s = cpool.tile([H2, 1], F32)
        w1f_s = cpool.tile([F_IN, H1], F32)
        w2af_s = cpool.tile([P, H2], F32)
        w2bf_s = cpool.tile([H1 - P, H2], F32)
        wlf_s = cpool.tile([H2, N_CLS], F32)
        maxnsl = max(m.maxnsl1, m.maxnsl2)
        gidx1_s = cpool.tile([P, m.slots1 * 8], I16)
        dstloc1_s = cpool.tile([P, m.slots1], BF16)
        gidx2_s = cpool.tile([P, m.slots2 * 8], I16)
        dstloc2_s = cpool.tile([P, m.slots2], BF16)
        iota_i = cpool.tile([P, DW], I32)
        iota_bf = cpool.tile([P, DW], BF16)
        iota3_bf = cpool.tile([P, maxnsl, DW], BF16)
        ident_f = cpool.tile([P, P], F32)
        identw_b = []
        for i in range(TPS):
            iwb = cpool.tile([P, DW], BF16, tag=f"iwb{i}")
            identw_b.append(iwb)
        dinvt_s = cpool.tile([P, NT], F32)
        dinvrep_s = cpool.tile([P, NT * P], BF16)
        sqdrow_s = cpool.tile([1, NT * P], BF16)
        b1row_s = cpool.tile([1, H1], BF16)
        b2row_s = cpool.tile([1, H2], BF16)
        ones_bf = cpool.tile([1, P], BF16)
        ones_s = cpool.tile([1, P], F32)
        gcnt1_s = cpool.tile([1, m.nb1], I32)
        gcnt2_s = cpool.tile([1, m.nb2], I32)

        nc.sync.dma_start(w1f_s[:], w1_d[:])
        nc.sync.dma_start(w2af_s[:], w2_d[0:P, :])
        nc.sync.dma_start(w2bf_s[:], w2_d[P:H1, :])
        nc.sync.dma_start(wlf_s[:], wl_d[:])
        nc.sync.dma_start(bl_s[:], bl_d[:])
        nc.sync.dma_start(b1a_s[:], b1_d[0:P, :])
        nc.sync.dma_start(b1b_s[:], b1_d[P:H1, :])
        nc.sync.dma_start(b2_s[:], b2_d[:])
        nc.sync.dma_start(gidx1_s[:], gidx1_d[:])
        nc.sync.dma_start(dstloc1_s[:], dstloc1_d[:])
        nc.sync.dma_start(gidx2_s[:], gidx2_d[:])
        nc.sync.dma_start(dstloc2_s[:], dstloc2_d[:])
        nc.sync.dma_start(dinvt_s[:], dinvt_d[:])
        nc.sync.dma_start(dinvrep_s[:], dinvrep_d[:])
        nc.sync.dma_start(sqdrow_s[:], sqdrow_d[:])
        nc.sync.dma_start(gcnt1_s[:], gcnt1_d[:])
        nc.sync.dma_start(gcnt2_s[:], gcnt2_d[:])
        nc.sync.dma_start(b1row_s[:], b1row_d[:])
        nc.sync.dma_start(b2row_s[:], b2row_d[:])

        nc.vector.tensor_copy(w1b_s[:], w1f_s[:])
        nc.vector.tensor_copy(w2ab_s[:], w2af_s[:])
        nc.vector.tensor_copy(w2bb_s[:], w2bf_s[:])
        nc.vector.tensor_copy(wlb_s[:], wlf_s[:])
        make_identity(nc, ident_f[:])
        for ti in range(TPS):
            nc.vector.memset(identw_b[ti][:], 0)
            nc.vector.tensor_copy(identw_b[ti][:, ti * P : (ti + 1) * P], ident_f[:])
        nc.gpsimd.iota(iota_i[:], [[1, DW]], channel_multiplier=0)
        nc.vector.tensor_copy(iota_bf[:], iota_i[:])
        for j in range(maxnsl):
            nc.vector.tensor_copy(iota3_bf[:, j, :], iota_bf[:])
        nc.vector.memset(ones_s[:], 1.0)
        nc.vector.memset(ones_bf[:], 1.0)

        # zero-fill msg buffers once: padded (skipped) gather lanes must read
        # finite values, never uninitialized SBUF.
        for i in range(MSG_BUFS):
            mz = mpool.tile([P, maxnsl, P], BF16, tag="msg")
            nc.vector.memset(mz[:], 0)

        # rotating registers for the per-bucket runtime gather counts: a
        # fresh register per gather keeps too many live at once (liveness
        # extends to the gather's DMA completion) and overflows the Pool
        # register file.
        cnt_regs = [
            nc.engines[mybir.EngineType.Pool].alloc_register(f"cntreg{i}")
            for i in range(16)
        ]
        creg = [0]

        def layer(sp_meta, table_aps, gidx_s, dstloc_s, cnt_s, feat,
                  epilogue, own_row, scat_bias=None):
            """own_row(t) -> (dram_ap, rows) for the self-loop tile load.

            scat_bias=(brow_s, srow_s): accumulate brow^T x srow[super cols]
            into scat (pre-divided bias, see l2 normalization)."""
            bi = [0]
            for sp in range(NS):
                groups = sp_meta[sp]
                ngroups = len(groups)
                last_g = groups[-1][1] + groups[-1][2] - 1 if ngroups else -1
                tiles = [
                    (t, min(P, chunk - t * P))
                    for t in range(sp * TPS, min(sp * TPS + TPS, NT))
                ]
                scat = scat_pp.tile([P, DW], F32, tag="scat")
                for ti, (t, rows) in enumerate(tiles):
                    src_ap, rows_ = own_row(t)
                    xt = wkpool.tile([P, P], BF16, tag="xt")
                    nc.sync.dma_start(xt[:rows_, :], src_ap)
                    nc.tensor.matmul(
                        out=scat[:feat, :],
                        lhsT=xt[:rows_, :feat],
                        rhs=identw_b[ti][:rows_, :],
                        start=(ti == 0),
                        stop=(scat_bias is None and ngroups == 0
                              and ti == len(tiles) - 1),
                    )
                if scat_bias is not None:
                    brow_s, srow_s = scat_bias
                    nc.tensor.matmul(
                        out=scat[:feat, :],
                        lhsT=brow_s[0:1, :feat],
                        rhs=srow_s[0:1, sp * DW : sp * DW + DW],
                        start=False,
                        stop=(ngroups == 0),
                    )
                for (q, slot0, nsl) in groups:
                    n_idx = nsl * P
                    if PAD_VALID:
                        cnt = n_idx
                    else:
                        r = cnt_regs[creg[0] % len(cnt_regs)]
                        creg[0] += 1
                        nc.engines[mybir.EngineType.Pool].reg_load(
                            r, cnt_s[0:1, bi[0] : bi[0] + 1]
                        )
                        cnt = nc.engines[mybir.EngineType.Pool].snap(
                            r, min_val=1, max_val=n_idx
                        )
                    bi[0] += 1
                    msg = mpool.tile([P, maxnsl, P], BF16, tag="msg")
                    nc.gpsimd.dma_gather(
                        out_ap=msg[:, 0:nsl, :],
                        in_ap=table_aps[q],
                        idxs_ap=gidx_s[:, slot0 * 8 : slot0 * 8 + n_idx // 16],
                        num_idxs=n_idx,
                        num_idxs_reg=cnt,
                        elem_size=P,
                        single_packet=(n_idx <= 1024),
                        queue_num=next_q(),
                    )
                    S = spool.tile([P, maxnsl, DW], BF16, tag="S")
                    nc.vector.tensor_tensor(
                        out=S[:, 0:nsl, :],
                        in0=iota3_bf[:, 0:nsl, :],
                        in1=dstloc_s[:, slot0 : slot0 + nsl]
                        .unsqueeze(2)
                        .to_broadcast([P, nsl, DW]),
                        op=mybir.AluOpType.is_equal,
                    )
                    for si in range(nsl):
                        g = slot0 + si
                        nc.tensor.matmul(
                            out=scat[:feat, :],
                            lhsT=msg[:, si, :feat],
                            rhs=S[:, si, :],
                            start=False,
                            stop=(g == last_g),
                        )
                for ti, (t, rows) in enumerate(tiles):
                    epilogue(t, scat[:feat, ti * P : (ti + 1) * P])

        def l1_epilogue(t, acc):
            rows = min(P, chunk - t * P)
            h1pre = wkpool.tile([P, P], BF16, tag="h1pre")
            nc.vector.tensor_tensor(
                out=h1pre[:],
                in0=acc,
                in1=dinvrep_s[:, t * P : (t + 1) * P],
                op=mybir.AluOpType.mult,
            )
            y1psa = y1_pp.tile([P, P], F32, tag="y1psa")
            y1psb = y1_pp.tile([H1 - P, P], F32, tag="y1psb")
            nc.tensor.matmul(
                out=y1psa[:], lhsT=w1b_s[:, 0:P], rhs=h1pre[:],
                start=True, stop=False,
            )
            nc.tensor.matmul(
                out=y1psa[:], lhsT=b1row_s[0:1, 0:P], rhs=ones_bf[0:1, :],
                start=False, stop=True,
            )
            nc.tensor.matmul(
                out=y1psb[:], lhsT=w1b_s[:, P:H1], rhs=h1pre[:],
                start=True, stop=False,
            )
            nc.tensor.matmul(
                out=y1psb[:], lhsT=b1row_s[0:1, P:H1],
                rhs=ones_bf[0:1, :], start=False, stop=True,
            )
            y1ua = wkpool.tile([P, P], F32, tag="y1ua")
            y1ub = wkpool.tile([H1 - P, P], F32, tag="y1ub")
            nc.scalar.copy(y1ua[:], y1psa[:])
            nc.scalar.copy(y1ub[:], y1psb[:])
            y1a = wkpool.tile([P, P], BF16, tag="y1a")
            y1b = wkpool.tile([H1 - P, P], BF16, tag="y1b")
            nc.vector.scalar_tensor_tensor(
                out=y1a[:], in0=y1ua[:], scalar=NEG_SLOPE, in1=y1ua[:],
                op0=mybir.AluOpType.mult, op1=mybir.AluOpType.max,
            )
            nc.vector.scalar_tensor_tensor(
                out=y1b[:], in0=y1ub[:], scalar=NEG_SLOPE, in1=y1ub[:],
                op0=mybir.AluOpType.mult, op1=mybir.AluOpType.max,
            )
            tps = t_pp.tile([P, H2], F32, tag="tps")
            nc.tensor.matmul(out=tps[:], lhsT=y1a[:], rhs=w2ab_s[:], start=True, stop=False)
            nc.tensor.matmul(out=tps[:], lhsT=y1b[:], rhs=w2bb_s[:], start=False, stop=True)
            t_sb = wkpool.tile([P, P], BF16, tag="t_sb")
            nc.scalar.activation(
                t_sb[:, 0:H2], tps[:], mybir.ActivationFunctionType.Copy,
                scale=dinvt_s[:, t : t + 1],
            )
            nc.vector.memset(t_sb[:, H2:P], 0)
            p, loc = tile_piece[t]
            nc.sync.dma_start(
                tchunk_p[p][loc : loc + rows, :], t_sb[:rows, :]
            )

        def l2_epilogue(t, acc):
            rows = min(P, chunk - t * P)
            y2_u = wkpool.tile([H2, P], F32, tag="y2_u")
            nc.vector.tensor_tensor(
                out=y2_u[:],
                in0=acc,
                in1=dinvrep_s[:H2, t * P : (t + 1) * P],
                op=mybir.AluOpType.mult,
            )
            y2 = wkpool.tile([H2, P], BF16, tag="y2")
            nc.vector.scalar_tensor_tensor(
                out=y2[:], in0=y2_u[:], scalar=NEG_SLOPE, in1=y2_u[:],
                op0=mybir.AluOpType.mult, op1=mybir.AluOpType.max,
            )
            lg = log_pp.tile([P, N_CLS], F32, tag="lg")
            nc.tensor.matmul(out=lg[:], lhsT=y2[:], rhs=wlb_s[:], start=True, stop=False)
            nc.tensor.matmul(out=lg[:], lhsT=ones_s[:], rhs=bl_s[:], start=False, stop=True)
            negm = wkpool.tile([P, 1], F32, tag="negm")
            nc.vector.tensor_reduce(
                negm[:], lg[:], mybir.AxisListType.X, mybir.AluOpType.max, negate=True
            )
            ex = wkpool.tile([P, N_CLS], F32, tag="ex")
            nc.scalar.activation(
                ex[:], lg[:], mybir.ActivationFunctionType.Exp,
                bias=negm[:, 0:1], scale=1.0,
            )
            ssum = wkpool.tile([P, 1], F32, tag="ssum")
            nc.vector.tensor_reduce(
                ssum[:], ex[:], mybir.AxisListType.X, mybir.AluOpType.add
            )
            lns = wkpool.tile([P, 1], F32, tag="lns")
            nc.scalar.activation(
                lns[:], ssum[:], mybir.ActivationFunctionType.Ln
            )
            negtot = wkpool.tile([P, 1], F32, tag="negtot")
            nc.vector.tensor_sub(negtot[:], negm[:], lns[:])
            osb = wkpool.tile([P, N_CLS], F32, tag="osb")
            nc.scalar.activation(
                osb[:], lg[:], mybir.ActivationFunctionType.Identity,
                bias=negtot[:, 0:1], scale=1.0,
            )
            nc.sync.dma_start(out_d[t * P : t * P + rows, :], osb[:rows, :])

        # ---- layer 1: aggregate raw x (bf16), transform to t, store by piece
        x_q = [xbf_d[lo:hi, :] for (lo, hi) in m.qbounds1]

        def l1_own(t):
            rows = min(P, chunk - t * P)
            return xownb_d[t * P : t * P + rows, :], rows

        layer(m.sp1, x_q, gidx1_s, dstloc1_s, gcnt1_s, F_IN,
              l1_epilogue, l1_own)

        # ---- piece-wise exchange of t
        if m.n_cores > 1:
            for p in range(m.NQ2):
                nc.gpsimd.collective_compute(
                    "AllGather",
                    mybir.AluOpType.bypass,
                    replica_groups=[list(range(m.n_cores))],
                    ins=[tchunk_p[p][:]],
                    outs=[tfull_p[p][:]],
                )
        else:
            for p in range(m.NQ2):
                nc.sync.dma_start(tfull_p[p][:], tchunk_p[p][:])

        # ---- layer 2: aggregate t by piece, epilogue -> log_softmax
        t_q = [tfull_p[p][:, :] for p in range(m.NQ2)]

        def l2_own(t):
            rows = min(P, chunk - t * P)
            p, loc = tile_piece[t]
            return tchunk_p[p][loc : loc + rows, :], rows

        layer(m.sp2, t_q, gidx2_s, dstloc2_s, gcnt2_s, H2,
              l2_epilogue, l2_own, scat_bias=(b2row_s, sqdrow_s))

    nc.compile()
    return nc


# ---------------------------------------------------------------- entry point

N_NODES = 100000
N_EDGES = 800000
N_CORES = 8

TRACE = False
LAST_EXEC_NS = None


def kernel(x, W1, b1, W2, b2, Wl, bl, edge_index):
    """Full-input GCN kernel: shards across 8 NeuronCores internally."""
    global LAST_EXEC_NS
    from concourse import bass_utils
    import ml_dtypes

    x = np.ascontiguousarray(np.asarray(x, dtype=np.float32))
    W1 = np.asarray(W1, dtype=np.float32)
    b1 = np.asarray(b1, dtype=np.float32).reshape(-1, 1)
    W2 = np.asarray(W2, dtype=np.float32)
    b2 = np.asarray(b2, dtype=np.float32).reshape(-1, 1)
    Wl = np.asarray(Wl, dtype=np.float32)
    bl = np.asarray(bl, dtype=np.float32).reshape(1, -1)
    edge_index = np.asarray(edge_index)

    n_nodes = x.shape[0]
    meta, per_core = prep(edge_index, n_nodes, n_cores=N_CORES)
    nc = build(meta)

    xbf = (x * meta.dinv[:, None]).astype(ml_dtypes.bfloat16)
    chunk = n_nodes // N_CORES
    shared = dict(xbf=xbf, W1=W1, b1=b1, W2=W2, b2=b2, Wl=Wl, bl=bl,
                  b1row=b1.reshape(1, -1).astype(ml_dtypes.bfloat16),
                  b2row=b2.reshape(1, -1).astype(ml_dtypes.bfloat16))
    in_maps = [
        {**shared, **{k: v for k, v in pc.items()},
         "xownb": xbf[k * chunk : (k + 1) * chunk]}
        for k, pc in enumerate(per_core)
    ]
    res = bass_utils.run_bass_kernel_spmd(
        nc, in_maps, core_ids=list(range(N_CORES)), trace=TRACE
    )
    LAST_EXEC_NS = res.exec_time_ns
    return np.concatenate([r["out"] for r in res.results], axis=0)
